# revision 22
# baseline (speedup 1.0000x reference)
"""GPT-NeoX attention layer as a Bass/Tile kernel for 8 Trainium2 NeuronCores.

Problem: hidden[2048,1,4096] -> QKV proj (W[4096,12288]) -> 32-head attention
(head_dim 128, rotary on first 32 dims, causal) -> dense proj (W[4096,4096]).

Sharding: tensor-parallel over heads (4 heads/core). hidden^T is replicated
to all cores by the host in fp16 (host prep also pre-permutes/casts weights),
so there is no on-device transpose phase and no hidden AllGather.

The kernel is a single fused pipeline built so the PE (the bottleneck at
~1.23M cycles) never starves:
  - QKV projection streams hidden^T once per s-block with the whole W_qkv
    shard SBUF-resident (read once). W_qkv columns are permuted head-major so
    each 128-row psum m-tile evacuates with one full-partition ACT copy;
    rotary is applied in-place on rows 0:32 of each head's q^T/k^T tile
    (rotate_half via a partition-permuting SBUF->SBUF DMA, sign baked into
    the sin table). q/k/v stored fp16.
  - attention for i-block ib is emitted INTERLEAVED into QKV block ib+1's
    matmul stream (a generator yields one small step per drain call), so its
    ACT-bound exp work hides under QKV matmuls. Heads run in pairs on
    ping-pong psum; scores^T tiles [kv 128 x q 512], additive causal mask on
    diagonal tiles, exp on ScalarE (fp16, scaled by 1/4 for fp16-sum
    headroom; no max-subtraction needed, scores are O(10)), denominator
    accumulated in fp16 on DVE, PV accumulates ctx^T in psum, normalization
    by 1/denom broadcast through a rank-1 matmul.
  - ctx^T AllGather fires per i-block as soon as its attention finishes
    (3 of 4 during the QKV phase).
  - the dense projection (column shard, fp16 out) runs last with attention
    i-block 3 drained between its k-steps; psum: 2-bank half-passes.
PSUM budget (8 banks): pv 1 + pq 1 + spA 1 + spB 1 + cp 2 + dn 1 + rb 1;
the dense phase reuses pv+pq's banks for its 2 half-pass accumulators.
Host gathers by concatenating the 8 column slices and casting to fp32.
"""
import sys
import os

sys.path.insert(0, "/opt/trn_rl_repo")

import numpy as np

import concourse.bacc as bacc
import concourse.mybir as mybir
import concourse.tile as tile

SEQ = 2048
HIDDEN = 4096
HEADS = 32
HD = 128
ROT = 32
HALF = ROT // 2  # 16
N_CORES = 8
HPC = HEADS // N_CORES       # 4 heads per core
CW = HPC * HD                # 512 columns of work per core (v / ctx / out)
KT = HIDDEN // 128           # 32 k-tiles over the hidden dim
SB = 512                     # sequence block
NSB = SEQ // SB              # 4
NST = SEQ // 128             # 16 sequence tiles
NEG = -32768.0               # additive mask value (pre-scale), fp16-exact
SCALE = float(1.0 / np.sqrt(HD))
EXP_BIAS = float(-2.0 * np.log(2.0))  # exp scaled by 1/4: fp16 sum headroom

F32 = mybir.dt.float32
F32R = mybir.dt.float32r
F16 = mybir.dt.float16
AF = mybir.ActivationFunctionType

_CACHE = {}


def _f32(ap):
    return ap.bitcast(F32)


def _build_program(rep=1, trace_sim=False, skip_cc=False, phases="all"):
    if phases in ("p01", "p1"):
        phases = "p1"
    elif phases in ("p012", "p12"):
        phases = "p12"
    nc = bacc.Bacc("TRN2", target_bir_lowering=False, debug=False,
                   num_devices=N_CORES)

    # ---- I/O ---------------------------------------------------------------
    # hidden^T, replicated, tiled [s-block, k-group, partition, k-in-group, s]
    hidT = nc.dram_tensor("hidT", [NSB, 8, 128, 4, SB], F16,
                          kind="ExternalInput")
    # w_qk: [m, p, k, c] fp16, head-major column permutation (see _host_prep)
    w_qk = nc.dram_tensor("w_qk", [8, 128, KT, 128], F16, kind="ExternalInput")
    w_v = nc.dram_tensor("w_v", [128, KT, CW], F16, kind="ExternalInput")
    w_d = nc.dram_tensor("w_d", [128, KT, CW], F16, kind="ExternalInput")
    b_qk = nc.dram_tensor("b_qk", [128, 8], F32, kind="ExternalInput")
    b_v = nc.dram_tensor("b_v", [1, CW], F16, kind="ExternalInput")
    b_d = nc.dram_tensor("b_d", [1, CW], F16, kind="ExternalInput")
    cos_in = nc.dram_tensor("cos_in", [ROT, SEQ], F16, kind="ExternalInput")
    sin_in = nc.dram_tensor("sin_in", [ROT, SEQ], F16, kind="ExternalInput")
    mask_in = nc.dram_tensor("mask_in", [128, 4 * SB], F16,
                             kind="ExternalInput")
    ebias_in = nc.dram_tensor("ebias_in", [128, 1], F32,
                              kind="ExternalInput")
    ones_col16_in = nc.dram_tensor("ones_col16_in", [128, 1], F16,
                                   kind="ExternalInput")
    ones_row_in = nc.dram_tensor("ones_row_in", [1, 128], F32R,
                                 kind="ExternalInput")
    ones_row16_in = nc.dram_tensor("ones_row16_in", [1, 128], F16,
                                   kind="ExternalInput")
    out = nc.dram_tensor("out", [SEQ, CW], F16, kind="ExternalOutput")

    rg = [list(range(N_CORES))]

    with tile.TileContext(nc, trace_sim=trace_sim) as tc:
        with (
            tc.tile_pool(name="const", bufs=1) as constp,
            tc.tile_pool(name="dram", bufs=1, space="DRAM") as dramp,
        ):
            # constants
            ones_col16 = constp.tile([128, 1], F16)
            ebias_sb = constp.tile([128, 1], F32)
            ones_row = constp.tile([1, 128], F32R)
            ones_row16 = constp.tile([1, 128], F16)
            bqk_sb = constp.tile([128, 8], F32)
            bv_sb = constp.tile([1, CW], F16)
            bd_sb = constp.tile([1, CW], F16)
            cos_sb = constp.tile([ROT, SEQ], F16)
            sin_sb = constp.tile([ROT, SEQ], F16)
            mask_sb = constp.tile([128, 4 * SB], F16)
            nc.sync.dma_start(ones_col16[:], ones_col16_in[:])
            nc.sync.dma_start(ebias_sb[:], ebias_in[:])
            nc.sync.dma_start(ones_row[:], ones_row_in[:])
            nc.sync.dma_start(ones_row16[:], ones_row16_in[:])
            nc.sync.dma_start(bqk_sb[:], b_qk[:])
            nc.sync.dma_start(bv_sb[:], b_v[:])
            nc.sync.dma_start(bd_sb[:], b_d[:])
            nc.sync.dma_start(cos_sb[:], cos_in[:])
            nc.sync.dma_start(sin_sb[:], sin_in[:])
            nc.sync.dma_start(mask_sb[:], mask_in[:])

            for _rep in range(rep):
              # collective bounce buffers, one per i-block so each AllGather
              # chunk can overlap compute (fresh per rep)
              ccin_ctx = [dramp.tile([CW, SB], F16, name=f"ccin_ctx{_rep}_{i}")
                          for i in range(NSB)]
              ccout_ctx = [dramp.tile([HIDDEN, SB], F16, addr_space="Shared",
                                      name=f"ccout_ctx{_rep}_{i}")
                           for i in range(NSB)]

              # persistent QKV outputs (live through the whole pipeline)
              with tc.tile_pool(name="qkvout", bufs=1) as qkvp:
                qh = [qkvp.tile([128, SEQ], F16, name=f"qh{h}")
                      for h in range(HPC)]
                kh = [qkvp.tile([128, SEQ], F16, name=f"kh{h}")
                      for h in range(HPC)]
                vsb = [qkvp.tile([128, CW], F16, name=f"v{s}")
                       for s in range(NST)]

                with (
                    tc.tile_pool(name="spp", bufs=1, space="PSUM") as spp,
                    tc.tile_pool(name="cps", bufs=2, space="PSUM") as cps,
                    tc.tile_pool(name="dps", bufs=1, space="PSUM") as dps,
                    tc.tile_pool(name="rbps", bufs=1, space="PSUM") as rbps,
                    tc.tile_pool(name="exp", bufs=5) as exp_p,
                    tc.tile_pool(name="accp", bufs=2) as accp,
                    tc.tile_pool(name="rcp", bufs=2) as rcp,
                    tc.tile_pool(name="rbp", bufs=1) as rbp,
                    tc.tile_pool(name="ctxp", bufs=1) as ctxp,
                ):
                    def attn_steps(ib):
                        """attention for i-block ib as a resumable stream of
                        small steps: one (head-pair, j-tile) scores/exp/PV
                        step per yield, then two tail steps (denominator,
                        normalize+ship), then the AllGather."""
                        icols = slice(ib * SB, (ib + 1) * SB)
                        njt = 4 * (ib + 1)
                        for hp in range(HPC // 2):
                            heads = (2 * hp, 2 * hp + 1)
                            cp = {h: cps.tile([128, SB], F32, name="cp")
                                  for h in heads}
                            acc = {h: accp.tile([128, SB], F16, name="acc")
                                   for h in heads}
                            ex0 = {}
                            ex_prev = None

                            def emit_pv(jt, exd):
                                for h in heads:
                                    e = exd[h]
                                    if jt == 0:
                                        ex0[h] = e
                                    elif jt == 1:
                                        nc.vector.tensor_add(
                                            acc[h][:], ex0[h][:], e[:])
                                    else:
                                        nc.vector.tensor_add(
                                            acc[h][:], acc[h][:], e[:])
                                    nc.tensor.matmul(
                                        cp[h][:],
                                        vsb[jt][:, h * 128:(h + 1) * 128],
                                        e[:], start=(jt == 0),
                                        stop=(jt == njt - 1))

                            for jt in range(njt):
                                ex_cur = {}
                                for i_h, h in enumerate(heads):
                                    sp = spp.tile(
                                        [128, SB], F32,
                                        name="spA" if i_h == 0 else "spB")
                                    nc.tensor.matmul(
                                        sp[:],
                                        kh[h][:, jt * 128:(jt + 1) * 128],
                                        qh[h][:, icols], start=True,
                                        stop=True)
                                    if jt >= 4 * ib:
                                        t = jt - 4 * ib
                                        nc.vector.tensor_add(
                                            sp[:], sp[:],
                                            mask_sb[:, t * SB:(t + 1) * SB])
                                    e = exp_p.tile([128, SB], F16, name="ex")
                                    nc.scalar.activation(
                                        e[:], sp[:], AF.Exp,
                                        bias=ebias_sb[:, 0:1], scale=SCALE)
                                    ex_cur[h] = e
                                # PV of the PREVIOUS j-tile: its exp has had
                                # a whole drain interval to complete
                                if ex_prev is not None:
                                    emit_pv(jt - 1, ex_prev)
                                ex_prev = ex_cur
                                yield True
                            emit_pv(njt - 1, ex_prev)
                            yield True
                            # tail A: denominators
                            rc = {}
                            for h in heads:
                                dn = dps.tile([1, SB], F32, name="dn")
                                nc.tensor.matmul(dn[:], ones_col16[:],
                                                 acc[h][:], start=True,
                                                 stop=True)
                                r = rcp.tile([1, SB], F32R, name="rc")
                                with nc.allow_low_precision(
                                        reason="f32r: 11-bit mantissa is "
                                               "plenty for the softmax "
                                               "denominator"):
                                    nc.vector.reciprocal(r[:], dn[:])
                                rc[h] = r
                            yield True
                            # tail B: broadcast 1/denom, normalize, ship
                            for h in heads:
                                rb = rbps.tile([128, SB], F32, name="rb")
                                nc.tensor.matmul(rb[:], ones_row[:],
                                                 rc[h][:], start=True,
                                                 stop=True)
                                rbs = rbp.tile([128, SB], F32R, name="rbs")
                                nc.scalar.activation(rbs[:], rb[:], AF.Copy)
                                ctxn = ctxp.tile([128, SB], F16, name="ctxn")
                                nc.vector.tensor_mul(ctxn[:], cp[h][:],
                                                     _f32(rbs[:]))
                                nc.gpsimd.dma_start(
                                    ccin_ctx[ib][h * 128:(h + 1) * 128, :],
                                    ctxn[:])
                            yield True
                        if not skip_cc and phases == "all":
                            nc.gpsimd.collective_compute(
                                "AllGather", mybir.AluOpType.bypass,
                                replica_groups=rg,
                                ins=[ccin_ctx[ib][:].opt()],
                                outs=[ccout_ctx[ib][:].opt()])
                        yield True

                    pending = []

                    def drain(n):
                        for _ in range(n):
                            while pending:
                                if next(pending[0], None) is not None:
                                    break
                                pending.pop(0)
                            else:
                                return

                    # ---- QKV (with attention riding along) ---------------
                    with (
                        tc.tile_pool(name="wres", bufs=1) as wres,
                        tc.tile_pool(name="htp", bufs=8) as htp,
                        tc.tile_pool(name="rscp", bufs=1) as rscp,
                        tc.tile_pool(name="vqps", bufs=1,
                                     space="PSUM") as vqps,
                    ):
                        # W shard resident, loaded once; order matters: v
                        # weights + first hidden block before q/k weights
                        wv_sb = wres.tile([128, KT * CW], F16, name="wv")
                        nc.sync.dma_start(wv_sb[:], w_v[:].opt())

                        def load_ht(sb):
                            tiles = []
                            for kg in range(8):
                                h4 = htp.tile([128, 4 * SB], F16, name="ht4")
                                nc.sync.dma_start(
                                    h4[:].rearrange("p (k s) -> p k s", k=4),
                                    hidT[sb, kg])
                                tiles.append(h4)
                            return tiles

                        ht4_first = load_ht(0)
                        wq_sb = [wres.tile([128, KT * 128], F16,
                                           name=f"wq{m}")
                                 for m in range(8)]
                        for m in range(8):
                            nc.sync.dma_start(wq_sb[m][:], w_qk[m].opt())

                        def rope(dst, sb):
                            """in-place partial rotary on rows 0:ROT of one
                            head's q^T/k^T tile; rotate_half materialized by
                            a partition-permuting SBUF->SBUF DMA, sign baked
                            into the sin table."""
                            scols = slice(sb * SB, (sb + 1) * SB)
                            shf = rscp.tile([ROT, SB], F16, name="shf")
                            nc.gpsimd.dma_start(shf[0:HALF, :],
                                                dst[HALF:ROT, scols])
                            nc.gpsimd.dma_start(shf[HALF:ROT, :],
                                                dst[0:HALF, scols])
                            t1 = rscp.tile([ROT, SB], F16, name="t1")
                            nc.vector.tensor_mul(t1[:], dst[0:ROT, scols],
                                                 cos_sb[:, scols])
                            nc.vector.tensor_mul(shf[:], shf[:],
                                                 sin_sb[:, scols])
                            nc.vector.tensor_add(dst[0:ROT, scols], t1[:],
                                                 shf[:])

                        def v_chunk(sb, ht4, q4):
                            def htk(k):
                                return ht4[k // 4][:, (k % 4) * SB:
                                                   (k % 4 + 1) * SB]
                            pv = vqps.tile([128, CW], F32, name="pv")
                            for k in range(KT):
                                nc.tensor.matmul(
                                    pv[:],
                                    htk(k)[:, q4 * 128:(q4 + 1) * 128],
                                    wv_sb[:, k * CW:(k + 1) * CW],
                                    start=(k == 0), stop=False)
                            nc.tensor.matmul(pv[:], ones_row16[:],
                                             bv_sb[:], start=False,
                                             stop=True)
                            nc.scalar.activation(vsb[sb * 4 + q4][:],
                                                 pv[:], AF.Copy)

                        def qk_chunk(sb, ht4, m):
                            def htk(k):
                                return ht4[k // 4][:, (k % 4) * SB:
                                                   (k % 4 + 1) * SB]
                            scols = slice(sb * SB, (sb + 1) * SB)
                            pq = vqps.tile([128, SB], F32, name="pq")
                            for k in range(KT):
                                nc.tensor.matmul(
                                    pq[:],
                                    wq_sb[m][:, k * 128:(k + 1) * 128],
                                    htk(k), start=(k == 0),
                                    stop=(k == KT - 1))
                            dst = qh[m % 4] if m < 4 else kh[m % 4]
                            nc.scalar.activation(dst[:, scols], pq[:],
                                                 AF.Identity,
                                                 bias=bqk_sb[:, m:m + 1])
                            rope(dst, sb)

                        for sb in range(NSB):
                            ht4 = ht4_first if sb == 0 else load_ht(sb)
                            if sb == 0:
                                # steady-state reps have next-rep weights
                                # preloaded during the prior dense phase, so
                                # block 0 can ping-pong too; v leads by two
                                # chunks to cover the first-rep weight DMAs
                                chunks = [("v", 0), ("v", 1), ("qk", 0),
                                          ("v", 2), ("qk", 1), ("v", 3)]
                                chunks += [("qk", m) for m in range(2, 8)]
                            else:
                                # ping-pong the two psum tags so each evac
                                # hides under the other tag's k-sweep
                                chunks = []
                                for i in range(4):
                                    chunks.append(("v", i))
                                    chunks.append(("qk", i))
                                chunks += [("qk", m) for m in range(4, 8)]
                            for kind, idx in chunks:
                                if kind == "v":
                                    v_chunk(sb, ht4, idx)
                                    drain(1)
                                else:
                                    qk_chunk(sb, ht4, idx)
                                    drain(2)

                            if phases != "p1" and sb < NSB - 1:
                                pending.append(attn_steps(sb))

                    # ---- tail: attention i-block 3 + dense ---------------
                    if phases == "p1":
                        nc.sync.dma_start(out[0:128, 0:SB], qh[0][:, 0:SB])
                        nc.sync.dma_start(out[128:256, :], vsb[0][:])
                    elif phases == "p12":
                        pending.append(attn_steps(NSB - 1))
                        drain(1000)
                        nc.sync.dma_start(out[0:CW, :], ccin_ctx[0][0:CW, :])
                    else:
                        pending.append(attn_steps(NSB - 1))
                        with (
                            tc.tile_pool(name="wdp", bufs=1) as wdp,
                            tc.tile_pool(name="ctp", bufs=10) as ctp,
                            tc.tile_pool(name="outp", bufs=3) as outp,
                            tc.tile_pool(name="pdps", bufs=2,
                                         space="PSUM") as pdps,
                        ):
                            wd_sb = wdp.tile([128, KT * CW], F16, name="wd")

                            def load_ct(mq, kb):
                                t = ctp.tile([128, 4 * SB], F16,
                                             name="ct4")
                                nc.sync.dma_start(
                                    t[:].rearrange(
                                        "p (k s) -> p k s", k=4),
                                    ccout_ctx[mq][kb * 512:
                                                  (kb + 1) * 512,
                                                  :].rearrange(
                                        "(k p) s -> p k s", k=4))
                                return t

                            def dense_block(mq):
                                if mq == 0:
                                    # first block: interleave the W_dense
                                    # chunks with the ct tiles in k-need
                                    # order, draining attention block 3
                                    # under the DMAs
                                    ct4 = [load_ct(0, 0)]
                                    for wc in range(4):
                                        nc.sync.dma_start(
                                            wd_sb[:, wc * 8 * CW:
                                                  (wc + 1) * 8 * CW],
                                            w_d[:, wc * 8:(wc + 1) * 8,
                                                :].opt())
                                        for kb in range(2 * wc + 1,
                                                        min(2 * wc + 3, 8)):
                                            ct4.append(load_ct(0, kb))
                                    drain(8)
                                else:
                                    ct4 = [load_ct(mq, kb)
                                           for kb in range(8)]
                                # 2-bank half-passes over the out m-tiles so
                                # attention block 3 drains between k-steps
                                for half in range(2):
                                    m4s = (2 * half, 2 * half + 1)
                                    pd = {m4: pdps.tile([128, CW], F32,
                                                        name="pd")
                                          for m4 in m4s}
                                    for k in range(KT):
                                        ct = ct4[k // 4][:, (k % 4) * SB:
                                                         (k % 4 + 1) * SB]
                                        for m4 in m4s:
                                            nc.tensor.matmul(
                                                pd[m4][:],
                                                ct[:, m4 * 128:
                                                   (m4 + 1) * 128],
                                                wd_sb[:, k * CW:
                                                      (k + 1) * CW],
                                                start=(k == 0), stop=False)
                                        if k % 2 == 0:
                                            drain(1)
                                    for m4 in m4s:
                                        nc.tensor.matmul(
                                            pd[m4][:], ones_row16[:],
                                            bd_sb[:], start=False,
                                            stop=True)
                                        ot = outp.tile([128, CW], F16,
                                                       name="ot")
                                        nc.scalar.activation(ot[:],
                                                             pd[m4][:],
                                                             AF.Copy)
                                        st = mq * 4 + m4
                                        nc.sync.dma_start(
                                            out[st * 128:(st + 1) * 128, :],
                                            ot[:])

                            for mq in range(4):
                                dense_block(mq)
                            drain(1000)

    nc.compile()
    return nc


def _get_exec(rep=1):
    if ("exec", rep) in _CACHE:
        return _CACHE[("exec", rep)]
    import jax
    from jax.sharding import Mesh, PartitionSpec
    from jax.experimental.shard_map import shard_map
    from concourse import bass2jax

    nc = _build_program(rep=rep)
    bass2jax.install_neuronx_cc_hook()

    partition_name = (nc.partition_id_tensor.name
                      if nc.partition_id_tensor else None)
    in_names = []
    out_names = []
    out_avals = []
    zero_shapes = []
    for alloc in nc.m.functions[0].allocations:
        if not isinstance(alloc, mybir.MemoryLocationSet):
            continue
        name = alloc.memorylocations[0].name
        if alloc.kind == "ExternalInput":
            if name != partition_name:
                in_names.append(name)
        elif alloc.kind == "ExternalOutput":
            np_dt = mybir.dt.np(alloc.dtype)
            out_names.append(name)
            out_avals.append(
                jax.core.ShapedArray(tuple(alloc.tensor_shape), np_dt))
            zero_shapes.append((tuple(alloc.tensor_shape), np_dt))

    n_params = len(in_names)
    n_outs = len(out_names)
    all_in_names = in_names + out_names
    if partition_name is not None:
        all_in_names = all_in_names + [partition_name]
    donate = tuple(range(n_params, n_params + n_outs))

    def _body(*args):
        operands = list(args)
        if partition_name is not None:
            operands.append(bass2jax.partition_id_tensor())
        outs = bass2jax._bass_exec_p.bind(
            *operands,
            out_avals=tuple(out_avals),
            in_names=tuple(all_in_names),
            out_names=tuple(out_names),
            lowering_input_output_aliases=(),
            sim_require_finite=True,
            sim_require_nnan=True,
            nc=nc,
        )
        return tuple(outs)

    devices = jax.devices()[:N_CORES]
    mesh = Mesh(np.asarray(devices), ("core",))
    in_specs = (PartitionSpec("core"),) * (n_params + n_outs)
    out_specs = (PartitionSpec("core"),) * n_outs
    sharded = jax.jit(
        shard_map(_body, mesh=mesh, in_specs=in_specs, out_specs=out_specs,
                  check_rep=False),
        donate_argnums=donate, keep_unused=True)

    _CACHE[("nc", rep)] = nc
    _CACHE[("exec", rep)] = (sharded, in_names, out_names, out_avals,
                             zero_shapes)
    return _CACHE[("exec", rep)]


def _run_cores(in_maps):
    """Run the SPMD program; in_maps is a list of 8 dicts name->np.ndarray."""
    sharded, in_names, out_names, out_avals, zero_shapes = _get_exec()
    concat_in = [
        np.concatenate([np.asarray(in_maps[c][n]) for c in range(N_CORES)],
                       axis=0)
        for n in in_names
    ]
    concat_zeros = [
        np.zeros((N_CORES * s[0], *s[1:]), dt) for (s, dt) in zero_shapes
    ]
    out_arrs = sharded(*concat_in, *concat_zeros)
    return [
        {n: np.asarray(out_arrs[i]).reshape(N_CORES, *out_avals[i].shape)[c]
         for i, n in enumerate(out_names)}
        for c in range(N_CORES)
    ]


def benchmark(in_maps, iters=10, rep=1):
    """Time repeated executions with device-resident inputs. Returns list of
    per-call wall seconds (axon RPC overhead included)."""
    import time
    import jax
    import jax.numpy as jnp
    from jax.sharding import Mesh, PartitionSpec, NamedSharding

    sharded, in_names, out_names, out_avals, zero_shapes = _get_exec(rep)
    devices = jax.devices()[:N_CORES]
    mesh = Mesh(np.asarray(devices), ("core",))
    shard = NamedSharding(mesh, PartitionSpec("core"))
    dev_in = [
        jax.device_put(
            np.concatenate([np.asarray(in_maps[c][n]) for c in range(N_CORES)],
                           axis=0), shard)
        for n in in_names
    ]
    jax.block_until_ready(dev_in)

    def make_zeros():
        zs = [jnp.zeros((N_CORES * s[0], *s[1:]), dt, device=shard)
              for (s, dt) in zero_shapes]
        jax.block_until_ready(zs)
        return zs

    out = sharded(*dev_in, *make_zeros())
    jax.block_until_ready(out)
    times = []
    for _ in range(iters):
        zs = make_zeros()
        t0 = time.perf_counter()
        out = sharded(*dev_in, *zs)
        jax.block_until_ready(out)
        times.append(time.perf_counter() - t0)
    return times


def _host_prep(hidden_states, W_qkv, b_qkv, W_dense, b_dense):
    hid = np.ascontiguousarray(
        np.asarray(hidden_states, dtype=np.float32).reshape(SEQ, HIDDEN))
    W_qkv = np.asarray(W_qkv, dtype=np.float32)
    b_qkv = np.asarray(b_qkv, dtype=np.float32)
    W_dense = np.asarray(W_dense, dtype=np.float32)
    b_dense = np.asarray(b_dense, dtype=np.float32)

    # hidden^T fp16, tiled [sb, kg, p, kk, s] (replicated to all cores)
    hidT16 = hid.T.astype(np.float16)                       # [4096, 2048]
    hidT_l = np.ascontiguousarray(
        hidT16.reshape(8, 4, 128, NSB, SB).transpose(3, 0, 2, 1, 4))

    # rotary tables, computed in float32 exactly as the reference does;
    # rows 0:16 carry -sin (rotate_half sign baked in)
    inv_freq = (1.0 / (np.float32(10000.0) **
                       (np.arange(0, ROT, 2, dtype=np.float32)
                        / np.float32(ROT))))
    t = np.arange(SEQ, dtype=np.float32)
    freqs = t[:, None] * inv_freq[None, :]          # [SEQ, 16]
    cosf = np.cos(freqs).T                          # [16, SEQ]
    sinf = np.sin(freqs).T
    cos_t = np.concatenate([cosf, cosf], axis=0).astype(np.float16)
    sin_t = np.concatenate([-sinf, sinf], axis=0).astype(np.float16)

    # additive causal masks for the 4 diagonal j-tiles of each i-block
    pj = np.arange(128)[:, None]
    fi = np.arange(SB)[None, :]
    mask = np.concatenate(
        [np.where(128 * t_ + pj <= fi, 0.0, NEG) for t_ in range(4)],
        axis=1).astype(np.float16)                   # [128, 4*SB]

    in_maps = []
    for c in range(N_CORES):
        heads = [HPC * c + i for i in range(HPC)]
        qcol = lambda h, d: h * 3 * HD + d
        kcol = lambda h, d: h * 3 * HD + HD + d
        vcol = lambda h, d: h * 3 * HD + 2 * HD + d
        # head-major: m-tile m < 4 is q of head m, m >= 4 is k of head m-4
        perm = [qcol(h, d) for h in heads for d in range(HD)]
        perm += [kcol(h, d) for h in heads for d in range(HD)]
        perm = np.asarray(perm)
        vperm = np.asarray([vcol(h, d) for h in heads for d in range(HD)])

        w_qk = W_qkv[:, perm].astype(np.float16)     # [4096, 1024]
        # [k, p, m, c] -> [m, p, k, c]
        w_qk = np.ascontiguousarray(
            w_qk.reshape(KT, 128, 8, 128).transpose(2, 1, 0, 3))
        w_v = np.ascontiguousarray(
            W_qkv[:, vperm].astype(np.float16).reshape(KT, 128, CW)
            .transpose(1, 0, 2))                     # [p, k, c]
        w_d = np.ascontiguousarray(
            W_dense[:, c * CW:(c + 1) * CW].astype(np.float16)
            .reshape(KT, 128, CW).transpose(1, 0, 2))
        in_maps.append({
            "hidT": hidT_l,
            "w_qk": w_qk,
            "w_v": w_v,
            "w_d": w_d,
            "b_qk": np.ascontiguousarray(b_qkv[perm].reshape(8, 128).T),
            "b_v": b_qkv[vperm].astype(np.float16).reshape(1, CW),
            "b_d": (b_dense[c * CW:(c + 1) * CW].astype(np.float16)
                    .reshape(1, CW)),
            "cos_in": cos_t,
            "sin_in": sin_t,
            "mask_in": mask,
            "ebias_in": np.full((128, 1), EXP_BIAS, np.float32),
            "ones_col16_in": np.ones((128, 1), np.float16),
            "ones_row_in": np.ones((1, 128), np.float32),
            "ones_row16_in": np.ones((1, 128), np.float16),
        })
    return in_maps


def kernel(hidden_states, attention_mask=None, W_qkv=None, b_qkv=None,
           W_dense=None, b_dense=None, **_unused):
    in_maps = _host_prep(hidden_states, W_qkv, b_qkv, W_dense, b_dense)
    results = _run_cores(in_maps)
    full = np.concatenate(
        [results[c]["out"].astype(np.float32) for c in range(N_CORES)],
        axis=1)
    return full.reshape(SEQ, 1, HIDDEN)


if __name__ == "__main__":
    rng = np.random.default_rng(0)
    ins = {
        "hidden_states": rng.standard_normal((SEQ, 1, HIDDEN),
                                             dtype=np.float32),
        "attention_mask": np.triu(np.ones((SEQ, SEQ), dtype=bool),
                                  1)[None, None],
        "W_qkv": (rng.standard_normal((HIDDEN, 3 * HIDDEN), dtype=np.float32)
                  * 0.02),
        "b_qkv": np.zeros(3 * HIDDEN, np.float32),
        "W_dense": (rng.standard_normal((HIDDEN, HIDDEN), dtype=np.float32)
                    * 0.02),
        "b_dense": np.zeros(HIDDEN, np.float32),
    }
    o = kernel(**ins)
    print("kernel output:", o.shape, o.dtype, float(np.abs(o).max()))


# revision 23
# speedup vs baseline: 1.0690x; 1.0690x over previous
"""GPT-NeoX attention layer as a Bass/Tile kernel for 8 Trainium2 NeuronCores.

Problem: hidden[2048,1,4096] -> QKV proj (W[4096,12288]) -> 32-head attention
(head_dim 128, rotary on first 32 dims, causal) -> dense proj (W[4096,4096]).

Sharding: tensor-parallel over heads (4 heads/core). hidden^T is replicated
to all cores by the host in fp16 (host prep also pre-permutes/casts weights),
so there is no on-device transpose phase and no hidden AllGather.

The kernel is a single fused pipeline built so the PE (the bottleneck at
~1.23M cycles) never starves:
  - QKV projection streams hidden^T once per s-block with the whole W_qkv
    shard SBUF-resident (read once). W_qkv columns are permuted head-major so
    each 128-row psum m-tile evacuates with one full-partition ACT copy;
    rotary is applied in-place on rows 0:32 of each head's q^T/k^T tile
    (rotate_half via a partition-permuting SBUF->SBUF DMA, sign baked into
    the sin table). q/k/v stored fp16.
  - attention for i-block ib is emitted INTERLEAVED into QKV block ib+1's
    matmul stream (a generator yields one small step per drain call), so its
    ACT-bound exp work hides under QKV matmuls. Heads run in pairs on
    ping-pong psum; scores^T tiles [kv 128 x q 512], additive causal mask on
    diagonal tiles, exp on ScalarE (fp16, scaled by 1/4 for fp16-sum
    headroom; no max-subtraction needed, scores are O(10)), denominator
    accumulated in fp16 on DVE, PV accumulates ctx^T in psum, normalization
    by 1/denom broadcast through a rank-1 matmul.
  - ctx^T AllGather fires per i-block as soon as its attention finishes
    (3 of 4 during the QKV phase).
  - the dense projection (column shard, fp16 out) runs last with attention
    i-block 3 drained between its k-steps; psum: 2-bank half-passes.
PSUM budget (8 banks): pv 1 + pq 1 + spA 1 + spB 1 + cp 2 + dn 1 + rb 1;
the dense phase reuses pv+pq's banks for its 2 half-pass accumulators.
Host gathers by concatenating the 8 column slices and casting to fp32.
"""
import sys
import os

sys.path.insert(0, "/opt/trn_rl_repo")

import numpy as np

import concourse.bacc as bacc
import concourse.mybir as mybir
import concourse.tile as tile

SEQ = 2048
HIDDEN = 4096
HEADS = 32
HD = 128
ROT = 32
HALF = ROT // 2  # 16
N_CORES = 8
HPC = HEADS // N_CORES       # 4 heads per core
CW = HPC * HD                # 512 columns of work per core (v / ctx / out)
KT = HIDDEN // 128           # 32 k-tiles over the hidden dim
SB = 512                     # sequence block
NSB = SEQ // SB              # 4
NST = SEQ // 128             # 16 sequence tiles
NEG = -32768.0               # additive mask value (pre-scale), fp16-exact
SCALE = float(1.0 / np.sqrt(HD))
EXP_BIAS = float(-2.0 * np.log(2.0))  # exp scaled by 1/4: fp16 sum headroom

F32 = mybir.dt.float32
F32R = mybir.dt.float32r
F16 = mybir.dt.float16
AF = mybir.ActivationFunctionType

_CACHE = {}


def _f32(ap):
    return ap.bitcast(F32)


def _build_program(rep=1, trace_sim=False, skip_cc=False, phases="all"):
    if phases in ("p01", "p1"):
        phases = "p1"
    elif phases in ("p012", "p12"):
        phases = "p12"
    nc = bacc.Bacc("TRN2", target_bir_lowering=False, debug=False,
                   num_devices=N_CORES)

    # ---- I/O ---------------------------------------------------------------
    # hidden^T, replicated, tiled [s-block, k-group, partition, k-in-group, s]
    hidT = nc.dram_tensor("hidT", [NSB, 8, 128, 4, SB], F16,
                          kind="ExternalInput")
    # w_qk: [m, p, k, c] fp16, head-major column permutation (see _host_prep)
    w_qk = nc.dram_tensor("w_qk", [8, 128, KT, 128], F16, kind="ExternalInput")
    w_v = nc.dram_tensor("w_v", [128, KT, CW], F16, kind="ExternalInput")
    w_d = nc.dram_tensor("w_d", [128, KT, CW], F16, kind="ExternalInput")
    b_qk = nc.dram_tensor("b_qk", [128, 8], F32, kind="ExternalInput")
    b_v = nc.dram_tensor("b_v", [1, CW], F16, kind="ExternalInput")
    b_d = nc.dram_tensor("b_d", [1, CW], F16, kind="ExternalInput")
    cos_in = nc.dram_tensor("cos_in", [ROT, SEQ], F16, kind="ExternalInput")
    sin_in = nc.dram_tensor("sin_in", [ROT, SEQ], F16, kind="ExternalInput")
    mask_in = nc.dram_tensor("mask_in", [128, 4 * SB], F16,
                             kind="ExternalInput")
    ebias_in = nc.dram_tensor("ebias_in", [128, 1], F32,
                              kind="ExternalInput")
    ones_col16_in = nc.dram_tensor("ones_col16_in", [128, 1], F16,
                                   kind="ExternalInput")
    ones_row_in = nc.dram_tensor("ones_row_in", [1, 128], F32R,
                                 kind="ExternalInput")
    ones_row16_in = nc.dram_tensor("ones_row16_in", [1, 128], F16,
                                   kind="ExternalInput")
    out = nc.dram_tensor("out", [SEQ, CW], F16, kind="ExternalOutput")

    rg = [list(range(N_CORES))]

    with tile.TileContext(nc, trace_sim=trace_sim) as tc:
        with (
            tc.tile_pool(name="const", bufs=1) as constp,
            tc.tile_pool(name="dram", bufs=1, space="DRAM") as dramp,
        ):
            # constants
            ones_col16 = constp.tile([128, 1], F16)
            ebias_sb = constp.tile([128, 1], F32)
            ones_row = constp.tile([1, 128], F32R)
            ones_row16 = constp.tile([1, 128], F16)
            bqk_sb = constp.tile([128, 8], F32)
            bv_sb = constp.tile([1, CW], F16)
            bd_sb = constp.tile([1, CW], F16)
            cos_sb = constp.tile([ROT, SEQ], F16)
            sin_sb = constp.tile([ROT, SEQ], F16)
            mask_sb = constp.tile([128, 4 * SB], F16)
            nc.sync.dma_start(ones_col16[:], ones_col16_in[:])
            nc.sync.dma_start(ebias_sb[:], ebias_in[:])
            nc.sync.dma_start(ones_row[:], ones_row_in[:])
            nc.sync.dma_start(ones_row16[:], ones_row16_in[:])
            nc.sync.dma_start(bqk_sb[:], b_qk[:])
            nc.sync.dma_start(bv_sb[:], b_v[:])
            nc.sync.dma_start(bd_sb[:], b_d[:])
            nc.sync.dma_start(cos_sb[:], cos_in[:])
            nc.sync.dma_start(sin_sb[:], sin_in[:])
            nc.sync.dma_start(mask_sb[:], mask_in[:])

            for _rep in range(rep):
              # collective bounce buffers, one per i-block so each AllGather
              # chunk can overlap compute (fresh per rep)
              ccin_ctx = [dramp.tile([CW, SB], F16, name=f"ccin_ctx{_rep}_{i}")
                          for i in range(NSB)]
              ccout_ctx = [dramp.tile([HIDDEN, SB], F16, addr_space="Shared",
                                      name=f"ccout_ctx{_rep}_{i}")
                           for i in range(NSB)]

              # persistent QKV outputs (live through the whole pipeline)
              with tc.tile_pool(name="qkvout", bufs=1) as qkvp:
                qh = [qkvp.tile([128, SEQ], F16, name=f"qh{h}")
                      for h in range(HPC)]
                kh = [qkvp.tile([128, SEQ], F16, name=f"kh{h}")
                      for h in range(HPC)]
                vsb = [qkvp.tile([128, CW], F16, name=f"v{s}")
                       for s in range(NST)]

                with (
                    tc.tile_pool(name="spp", bufs=1, space="PSUM") as spp,
                    tc.tile_pool(name="cps", bufs=2, space="PSUM") as cps,
                    tc.tile_pool(name="dps", bufs=1, space="PSUM") as dps,
                    tc.tile_pool(name="rbps", bufs=1, space="PSUM") as rbps,
                    tc.tile_pool(name="exp", bufs=4) as exp_p,
                    tc.tile_pool(name="accp", bufs=2) as accp,
                    tc.tile_pool(name="rcp", bufs=1) as rcp,
                    tc.tile_pool(name="rbp", bufs=1) as rbp,
                    tc.tile_pool(name="ctxp", bufs=1) as ctxp,
                ):
                    def attn_steps(ib):
                        """attention for i-block ib as a resumable stream of
                        small steps: one (head-pair, j-tile) scores/exp/PV
                        step per yield, then two tail steps (denominator,
                        normalize+ship), then the AllGather."""
                        icols = slice(ib * SB, (ib + 1) * SB)
                        njt = 4 * (ib + 1)
                        for hp in range(HPC // 2):
                            heads = (2 * hp, 2 * hp + 1)
                            cp = {h: cps.tile([128, SB], F32, name="cp")
                                  for h in heads}
                            acc = {h: accp.tile([128, SB], F16, name="acc")
                                   for h in heads}
                            ex0 = {}
                            ex_prev = None

                            def emit_pv(jt, exd):
                                for h in heads:
                                    e = exd[h]
                                    if jt == 0:
                                        ex0[h] = e
                                    elif jt == 1:
                                        nc.vector.tensor_add(
                                            acc[h][:], ex0[h][:], e[:])
                                    else:
                                        nc.vector.tensor_add(
                                            acc[h][:], acc[h][:], e[:])
                                    nc.tensor.matmul(
                                        cp[h][:],
                                        vsb[jt][:, h * 128:(h + 1) * 128],
                                        e[:], start=(jt == 0),
                                        stop=(jt == njt - 1))

                            for jt in range(njt):
                                ex_cur = {}
                                for i_h, h in enumerate(heads):
                                    sp = spp.tile(
                                        [128, SB], F32,
                                        name="spA" if i_h == 0 else "spB")
                                    nc.tensor.matmul(
                                        sp[:],
                                        kh[h][:, jt * 128:(jt + 1) * 128],
                                        qh[h][:, icols], start=True,
                                        stop=True)
                                    if jt >= 4 * ib:
                                        t = jt - 4 * ib
                                        nc.vector.tensor_add(
                                            sp[:], sp[:],
                                            mask_sb[:, t * SB:(t + 1) * SB])
                                    e = exp_p.tile([128, SB], F16, name="ex")
                                    nc.scalar.activation(
                                        e[:], sp[:], AF.Exp,
                                        bias=ebias_sb[:, 0:1], scale=SCALE)
                                    ex_cur[h] = e
                                # PV of the PREVIOUS j-tile: its exp has had
                                # a whole drain interval to complete
                                if ex_prev is not None:
                                    emit_pv(jt - 1, ex_prev)
                                ex_prev = ex_cur
                                yield True
                            emit_pv(njt - 1, ex_prev)
                            yield True
                            # tail A: denominators
                            rc = {}
                            for h in heads:
                                dn = dps.tile([1, SB], F32, name="dn")
                                nc.tensor.matmul(dn[:], ones_col16[:],
                                                 acc[h][:], start=True,
                                                 stop=True)
                                r = rcp.tile([1, SB], F32R, name="rc")
                                with nc.allow_low_precision(
                                        reason="f32r: 11-bit mantissa is "
                                               "plenty for the softmax "
                                               "denominator"):
                                    nc.vector.reciprocal(r[:], dn[:])
                                rc[h] = r
                            yield True
                            # tail B: broadcast 1/denom, normalize, ship
                            for h in heads:
                                rb = rbps.tile([128, SB], F32, name="rb")
                                nc.tensor.matmul(rb[:], ones_row[:],
                                                 rc[h][:], start=True,
                                                 stop=True)
                                rbs = rbp.tile([128, SB], F32R, name="rbs")
                                nc.scalar.activation(rbs[:], rb[:], AF.Copy)
                                ctxn = ctxp.tile([128, SB], F16, name="ctxn")
                                nc.vector.tensor_mul(ctxn[:], cp[h][:],
                                                     _f32(rbs[:]))
                                nc.gpsimd.dma_start(
                                    ccin_ctx[ib][h * 128:(h + 1) * 128, :],
                                    ctxn[:])
                            yield True
                        if not skip_cc and phases == "all":
                            nc.gpsimd.collective_compute(
                                "AllGather", mybir.AluOpType.bypass,
                                replica_groups=rg,
                                ins=[ccin_ctx[ib][:].opt()],
                                outs=[ccout_ctx[ib][:].opt()])
                        yield True

                    pending = []

                    def drain(n):
                        for _ in range(n):
                            while pending:
                                if next(pending[0], None) is not None:
                                    break
                                pending.pop(0)
                            else:
                                return

                    # ---- QKV (with attention riding along) ---------------
                    with (
                        tc.tile_pool(name="wres", bufs=1) as wres,
                        tc.tile_pool(name="htp", bufs=9) as htp,
                        tc.tile_pool(name="rscp", bufs=1) as rscp,
                        tc.tile_pool(name="vqps", bufs=1,
                                     space="PSUM") as vqps,
                    ):
                        # W shard resident, loaded once; order matters: v
                        # weights + first hidden block before q/k weights
                        wv_sb = wres.tile([128, KT * CW], F16, name="wv")
                        nc.sync.dma_start(wv_sb[:], w_v[:].opt())

                        def load_ht(sb):
                            tiles = []
                            for kg in range(8):
                                h4 = htp.tile([128, 4 * SB], F16, name="ht4")
                                nc.sync.dma_start(
                                    h4[:].rearrange("p (k s) -> p k s", k=4),
                                    hidT[sb, kg])
                                tiles.append(h4)
                            return tiles

                        ht4_first = load_ht(0)
                        wq_sb = [wres.tile([128, KT * 128], F16,
                                           name=f"wq{m}")
                                 for m in range(8)]
                        for m in range(8):
                            nc.sync.dma_start(wq_sb[m][:], w_qk[m].opt())

                        def rope(dst, sb):
                            """in-place partial rotary on rows 0:ROT of one
                            head's q^T/k^T tile; rotate_half materialized by
                            a partition-permuting SBUF->SBUF DMA, sign baked
                            into the sin table."""
                            scols = slice(sb * SB, (sb + 1) * SB)
                            shf = rscp.tile([ROT, SB], F16, name="shf")
                            nc.gpsimd.dma_start(shf[0:HALF, :],
                                                dst[HALF:ROT, scols])
                            nc.gpsimd.dma_start(shf[HALF:ROT, :],
                                                dst[0:HALF, scols])
                            t1 = rscp.tile([ROT, SB], F16, name="t1")
                            nc.vector.tensor_mul(t1[:], dst[0:ROT, scols],
                                                 cos_sb[:, scols])
                            nc.vector.tensor_mul(shf[:], shf[:],
                                                 sin_sb[:, scols])
                            nc.vector.tensor_add(dst[0:ROT, scols], t1[:],
                                                 shf[:])

                        def v_chunk(sb, ht4, q4):
                            def htk(k):
                                return ht4[k // 4][:, (k % 4) * SB:
                                                   (k % 4 + 1) * SB]
                            pv = vqps.tile([128, CW], F32, name="pv")
                            for k in range(KT):
                                nc.tensor.matmul(
                                    pv[:],
                                    htk(k)[:, q4 * 128:(q4 + 1) * 128],
                                    wv_sb[:, k * CW:(k + 1) * CW],
                                    start=(k == 0), stop=False)
                            nc.tensor.matmul(pv[:], ones_row16[:],
                                             bv_sb[:], start=False,
                                             stop=True)
                            nc.scalar.activation(vsb[sb * 4 + q4][:],
                                                 pv[:], AF.Copy)

                        def qk_chunk(sb, ht4, m):
                            def htk(k):
                                return ht4[k // 4][:, (k % 4) * SB:
                                                   (k % 4 + 1) * SB]
                            scols = slice(sb * SB, (sb + 1) * SB)
                            pq = vqps.tile([128, SB], F32, name="pq")
                            for k in range(KT):
                                nc.tensor.matmul(
                                    pq[:],
                                    wq_sb[m][:, k * 128:(k + 1) * 128],
                                    htk(k), start=(k == 0),
                                    stop=(k == KT - 1))
                            dst = qh[m % 4] if m < 4 else kh[m % 4]
                            nc.scalar.activation(dst[:, scols], pq[:],
                                                 AF.Identity,
                                                 bias=bqk_sb[:, m:m + 1])
                            rope(dst, sb)

                        for sb in range(NSB):
                            ht4 = ht4_first if sb == 0 else load_ht(sb)
                            if sb == 0:
                                # steady-state reps have next-rep weights
                                # preloaded during the prior dense phase, so
                                # block 0 can ping-pong too; v leads by two
                                # chunks to cover the first-rep weight DMAs
                                chunks = [("v", 0), ("v", 1), ("qk", 0),
                                          ("v", 2), ("qk", 1), ("v", 3)]
                                chunks += [("qk", m) for m in range(2, 8)]
                            else:
                                # ping-pong the two psum tags so each evac
                                # hides under the other tag's k-sweep
                                chunks = []
                                for i in range(4):
                                    chunks.append(("v", i))
                                    chunks.append(("qk", i))
                                chunks += [("qk", m) for m in range(4, 8)]
                            for kind, idx in chunks:
                                if kind == "v":
                                    v_chunk(sb, ht4, idx)
                                    drain(1)
                                else:
                                    qk_chunk(sb, ht4, idx)
                                    drain(2)

                            if phases != "p1" and sb < NSB - 1:
                                pending.append(attn_steps(sb))

                    # ---- tail: attention i-block 3 + dense ---------------
                    if phases == "p1":
                        nc.sync.dma_start(out[0:128, 0:SB], qh[0][:, 0:SB])
                        nc.sync.dma_start(out[128:256, :], vsb[0][:])
                    elif phases == "p12":
                        pending.append(attn_steps(NSB - 1))
                        drain(1000)
                        nc.sync.dma_start(out[0:CW, :], ccin_ctx[0][0:CW, :])
                    else:
                        pending.append(attn_steps(NSB - 1))
                        with (
                            tc.tile_pool(name="wdp", bufs=1) as wdp,
                            tc.tile_pool(name="ctp", bufs=10) as ctp,
                            tc.tile_pool(name="outp", bufs=3) as outp,
                            tc.tile_pool(name="pdps", bufs=2,
                                         space="PSUM") as pdps,
                        ):
                            wd_sb = wdp.tile([128, KT * CW], F16, name="wd")

                            def load_ct(mq, kb):
                                t = ctp.tile([128, 4 * SB], F16,
                                             name="ct4")
                                nc.sync.dma_start(
                                    t[:].rearrange(
                                        "p (k s) -> p k s", k=4),
                                    ccout_ctx[mq][kb * 512:
                                                  (kb + 1) * 512,
                                                  :].rearrange(
                                        "(k p) s -> p k s", k=4))
                                return t

                            def dense_block(mq):
                                if mq == 0:
                                    # first block: interleave the W_dense
                                    # chunks with the ct tiles in k-need
                                    # order, draining attention block 3
                                    # under the DMAs
                                    ct4 = [load_ct(0, 0)]
                                    for wc in range(4):
                                        nc.sync.dma_start(
                                            wd_sb[:, wc * 8 * CW:
                                                  (wc + 1) * 8 * CW],
                                            w_d[:, wc * 8:(wc + 1) * 8,
                                                :].opt())
                                        for kb in range(2 * wc + 1,
                                                        min(2 * wc + 3, 8)):
                                            ct4.append(load_ct(0, kb))
                                    drain(8)
                                else:
                                    ct4 = [load_ct(mq, kb)
                                           for kb in range(8)]
                                # 2-bank half-passes over the out m-tiles so
                                # attention block 3 drains between k-steps
                                for half in range(2):
                                    m4s = (2 * half, 2 * half + 1)
                                    pd = {m4: pdps.tile([128, CW], F32,
                                                        name="pd")
                                          for m4 in m4s}
                                    for k in range(KT):
                                        ct = ct4[k // 4][:, (k % 4) * SB:
                                                         (k % 4 + 1) * SB]
                                        for m4 in m4s:
                                            nc.tensor.matmul(
                                                pd[m4][:],
                                                ct[:, m4 * 128:
                                                   (m4 + 1) * 128],
                                                wd_sb[:, k * CW:
                                                      (k + 1) * CW],
                                                start=(k == 0), stop=False)
                                        if k % 2 == 0:
                                            drain(1)
                                    for m4 in m4s:
                                        nc.tensor.matmul(
                                            pd[m4][:], ones_row16[:],
                                            bd_sb[:], start=False,
                                            stop=True)
                                        ot = outp.tile([128, CW], F16,
                                                       name="ot")
                                        nc.scalar.activation(ot[:],
                                                             pd[m4][:],
                                                             AF.Copy)
                                        st = mq * 4 + m4
                                        nc.sync.dma_start(
                                            out[st * 128:(st + 1) * 128, :],
                                            ot[:])

                            for mq in range(4):
                                dense_block(mq)
                            drain(1000)

    nc.compile()
    return nc


def _get_exec(rep=1):
    if ("exec", rep) in _CACHE:
        return _CACHE[("exec", rep)]
    import jax
    from jax.sharding import Mesh, PartitionSpec
    from jax.experimental.shard_map import shard_map
    from concourse import bass2jax

    nc = _build_program(rep=rep)
    bass2jax.install_neuronx_cc_hook()

    partition_name = (nc.partition_id_tensor.name
                      if nc.partition_id_tensor else None)
    in_names = []
    out_names = []
    out_avals = []
    zero_shapes = []
    for alloc in nc.m.functions[0].allocations:
        if not isinstance(alloc, mybir.MemoryLocationSet):
            continue
        name = alloc.memorylocations[0].name
        if alloc.kind == "ExternalInput":
            if name != partition_name:
                in_names.append(name)
        elif alloc.kind == "ExternalOutput":
            np_dt = mybir.dt.np(alloc.dtype)
            out_names.append(name)
            out_avals.append(
                jax.core.ShapedArray(tuple(alloc.tensor_shape), np_dt))
            zero_shapes.append((tuple(alloc.tensor_shape), np_dt))

    n_params = len(in_names)
    n_outs = len(out_names)
    all_in_names = in_names + out_names
    if partition_name is not None:
        all_in_names = all_in_names + [partition_name]
    donate = tuple(range(n_params, n_params + n_outs))

    def _body(*args):
        operands = list(args)
        if partition_name is not None:
            operands.append(bass2jax.partition_id_tensor())
        outs = bass2jax._bass_exec_p.bind(
            *operands,
            out_avals=tuple(out_avals),
            in_names=tuple(all_in_names),
            out_names=tuple(out_names),
            lowering_input_output_aliases=(),
            sim_require_finite=True,
            sim_require_nnan=True,
            nc=nc,
        )
        return tuple(outs)

    devices = jax.devices()[:N_CORES]
    mesh = Mesh(np.asarray(devices), ("core",))
    in_specs = (PartitionSpec("core"),) * (n_params + n_outs)
    out_specs = (PartitionSpec("core"),) * n_outs
    sharded = jax.jit(
        shard_map(_body, mesh=mesh, in_specs=in_specs, out_specs=out_specs,
                  check_rep=False),
        donate_argnums=donate, keep_unused=True)

    _CACHE[("nc", rep)] = nc
    _CACHE[("exec", rep)] = (sharded, in_names, out_names, out_avals,
                             zero_shapes)
    return _CACHE[("exec", rep)]


def _run_cores(in_maps):
    """Run the SPMD program; in_maps is a list of 8 dicts name->np.ndarray."""
    sharded, in_names, out_names, out_avals, zero_shapes = _get_exec()
    concat_in = [
        np.concatenate([np.asarray(in_maps[c][n]) for c in range(N_CORES)],
                       axis=0)
        for n in in_names
    ]
    concat_zeros = [
        np.zeros((N_CORES * s[0], *s[1:]), dt) for (s, dt) in zero_shapes
    ]
    out_arrs = sharded(*concat_in, *concat_zeros)
    return [
        {n: np.asarray(out_arrs[i]).reshape(N_CORES, *out_avals[i].shape)[c]
         for i, n in enumerate(out_names)}
        for c in range(N_CORES)
    ]


def benchmark(in_maps, iters=10, rep=1):
    """Time repeated executions with device-resident inputs. Returns list of
    per-call wall seconds (axon RPC overhead included)."""
    import time
    import jax
    import jax.numpy as jnp
    from jax.sharding import Mesh, PartitionSpec, NamedSharding

    sharded, in_names, out_names, out_avals, zero_shapes = _get_exec(rep)
    devices = jax.devices()[:N_CORES]
    mesh = Mesh(np.asarray(devices), ("core",))
    shard = NamedSharding(mesh, PartitionSpec("core"))
    dev_in = [
        jax.device_put(
            np.concatenate([np.asarray(in_maps[c][n]) for c in range(N_CORES)],
                           axis=0), shard)
        for n in in_names
    ]
    jax.block_until_ready(dev_in)

    def make_zeros():
        zs = [jnp.zeros((N_CORES * s[0], *s[1:]), dt, device=shard)
              for (s, dt) in zero_shapes]
        jax.block_until_ready(zs)
        return zs

    out = sharded(*dev_in, *make_zeros())
    jax.block_until_ready(out)
    times = []
    for _ in range(iters):
        zs = make_zeros()
        t0 = time.perf_counter()
        out = sharded(*dev_in, *zs)
        jax.block_until_ready(out)
        times.append(time.perf_counter() - t0)
    return times


def _host_prep(hidden_states, W_qkv, b_qkv, W_dense, b_dense):
    hid = np.ascontiguousarray(
        np.asarray(hidden_states, dtype=np.float32).reshape(SEQ, HIDDEN))
    W_qkv = np.asarray(W_qkv, dtype=np.float32)
    b_qkv = np.asarray(b_qkv, dtype=np.float32)
    W_dense = np.asarray(W_dense, dtype=np.float32)
    b_dense = np.asarray(b_dense, dtype=np.float32)

    # hidden^T fp16, tiled [sb, kg, p, kk, s] (replicated to all cores)
    hidT16 = hid.T.astype(np.float16)                       # [4096, 2048]
    hidT_l = np.ascontiguousarray(
        hidT16.reshape(8, 4, 128, NSB, SB).transpose(3, 0, 2, 1, 4))

    # rotary tables, computed in float32 exactly as the reference does;
    # rows 0:16 carry -sin (rotate_half sign baked in)
    inv_freq = (1.0 / (np.float32(10000.0) **
                       (np.arange(0, ROT, 2, dtype=np.float32)
                        / np.float32(ROT))))
    t = np.arange(SEQ, dtype=np.float32)
    freqs = t[:, None] * inv_freq[None, :]          # [SEQ, 16]
    cosf = np.cos(freqs).T                          # [16, SEQ]
    sinf = np.sin(freqs).T
    cos_t = np.concatenate([cosf, cosf], axis=0).astype(np.float16)
    sin_t = np.concatenate([-sinf, sinf], axis=0).astype(np.float16)

    # additive causal masks for the 4 diagonal j-tiles of each i-block
    pj = np.arange(128)[:, None]
    fi = np.arange(SB)[None, :]
    mask = np.concatenate(
        [np.where(128 * t_ + pj <= fi, 0.0, NEG) for t_ in range(4)],
        axis=1).astype(np.float16)                   # [128, 4*SB]

    in_maps = []
    for c in range(N_CORES):
        heads = [HPC * c + i for i in range(HPC)]
        qcol = lambda h, d: h * 3 * HD + d
        kcol = lambda h, d: h * 3 * HD + HD + d
        vcol = lambda h, d: h * 3 * HD + 2 * HD + d
        # head-major: m-tile m < 4 is q of head m, m >= 4 is k of head m-4
        perm = [qcol(h, d) for h in heads for d in range(HD)]
        perm += [kcol(h, d) for h in heads for d in range(HD)]
        perm = np.asarray(perm)
        vperm = np.asarray([vcol(h, d) for h in heads for d in range(HD)])

        w_qk = W_qkv[:, perm].astype(np.float16)     # [4096, 1024]
        # [k, p, m, c] -> [m, p, k, c]
        w_qk = np.ascontiguousarray(
            w_qk.reshape(KT, 128, 8, 128).transpose(2, 1, 0, 3))
        w_v = np.ascontiguousarray(
            W_qkv[:, vperm].astype(np.float16).reshape(KT, 128, CW)
            .transpose(1, 0, 2))                     # [p, k, c]
        w_d = np.ascontiguousarray(
            W_dense[:, c * CW:(c + 1) * CW].astype(np.float16)
            .reshape(KT, 128, CW).transpose(1, 0, 2))
        in_maps.append({
            "hidT": hidT_l,
            "w_qk": w_qk,
            "w_v": w_v,
            "w_d": w_d,
            "b_qk": np.ascontiguousarray(b_qkv[perm].reshape(8, 128).T),
            "b_v": b_qkv[vperm].astype(np.float16).reshape(1, CW),
            "b_d": (b_dense[c * CW:(c + 1) * CW].astype(np.float16)
                    .reshape(1, CW)),
            "cos_in": cos_t,
            "sin_in": sin_t,
            "mask_in": mask,
            "ebias_in": np.full((128, 1), EXP_BIAS, np.float32),
            "ones_col16_in": np.ones((128, 1), np.float16),
            "ones_row_in": np.ones((1, 128), np.float32),
            "ones_row16_in": np.ones((1, 128), np.float16),
        })
    return in_maps


def kernel(hidden_states, attention_mask=None, W_qkv=None, b_qkv=None,
           W_dense=None, b_dense=None, **_unused):
    in_maps = _host_prep(hidden_states, W_qkv, b_qkv, W_dense, b_dense)
    results = _run_cores(in_maps)
    full = np.concatenate(
        [results[c]["out"].astype(np.float32) for c in range(N_CORES)],
        axis=1)
    return full.reshape(SEQ, 1, HIDDEN)


if __name__ == "__main__":
    rng = np.random.default_rng(0)
    ins = {
        "hidden_states": rng.standard_normal((SEQ, 1, HIDDEN),
                                             dtype=np.float32),
        "attention_mask": np.triu(np.ones((SEQ, SEQ), dtype=bool),
                                  1)[None, None],
        "W_qkv": (rng.standard_normal((HIDDEN, 3 * HIDDEN), dtype=np.float32)
                  * 0.02),
        "b_qkv": np.zeros(3 * HIDDEN, np.float32),
        "W_dense": (rng.standard_normal((HIDDEN, HIDDEN), dtype=np.float32)
                    * 0.02),
        "b_dense": np.zeros(HIDDEN, np.float32),
    }
    o = kernel(**ins)
    print("kernel output:", o.shape, o.dtype, float(np.abs(o).max()))
